# revision 3
# baseline (speedup 1.0000x reference)
"""Distributed Trainium2 kernel for nn_Attention_49529562858354.

Reference computation (per batch): LayerNorm(x) @ w_qkv -> 16-head
self-attention with key-side masking (mask==1 -> key excluded).

Sharding (8 cores): core = batch * 2 + head_group. Data parallel over
the 4 batches, tensor parallel over 2 groups of 8 heads. Each core gets
its batch's x, the w_qkv column slice for its heads, and produces
out[:, hg*512:(hg+1)*512] for its batch. No collectives needed.

Per-core pipeline:
  1. LayerNorm stats in natural layout (bn_stats), xhat -> bf16.
  2. xhat -> DRAM scratch -> 8 DMA-transposes -> xT [128d x 8, tokens].
     ln_g/ln_b applied per-partition on xT (d is the partition dim there).
  3. QKV projections in bf16: qT/kT as [cols, tokens] (transposed),
     v in natural [tokens, cols] layout with the key gate folded in and
     a gate column appended (softmax denominator via matmul).
  4. Attention per (head, q-chunk of 512): scores computed transposed
     [k_tile=128, 512q] into PSUM, exp on the Scalar engine directly
     from PSUM (scale fused, no max subtraction needed: logits are O(1)
     after LN), AV accumulates outT_aug [65, 512] whose row 64 is the
     softmax denominator. Epilogue: PE transpose, reciprocal, scale,
     assemble [128, 512] output tiles, DMA out.

Masked keys are removed on the host (gather) and the remainder padded
to a multiple of 128 with gate=0 rows, halving attention work; the same
kernel runs dense (all 2048 keys, gate = 1-mask) when COMPACT=False.
"""

import os
import sys
import types

for _p in ("/opt/trn_rl_repo", "/root/.axon_site"):
    if _p not in sys.path:
        sys.path.insert(0, _p)

import numpy as np
import ml_dtypes

import concourse.bass as bass
import concourse.tile as tile
from concourse import mybir

N_CORES = 8
N_TOK = 2048
DIM = 1024
HEADS_LOCAL = 8
DH = 64
COLS = HEADS_LOCAL * DH  # 512 columns per core per q/k/v
SCALE = DH ** -0.5
EPS = 1e-5
QCHUNK = 512
KGROUP = 3  # score k-tiles per PSUM group / exp call
COMPACT = os.environ.get("KERNEL_DENSE", "") != "1"

F32 = mybir.dt.float32
BF16 = mybir.dt.bfloat16

LAST_EXEC_TIME_NS = None


def _split_excess_waits(nc, max_waits=1, max_updates=1):
    """This container's walrus rejects >1 sync wait/update per
    instruction; move overflow onto adjacent same-engine NoOps."""
    counter = [0]

    def fresh():
        counter[0] += 1
        return f"I-WFIX-{counter[0]}"

    for f in nc.m.functions:
        for blk in f.blocks:
            il = blk.instructions
            out = []
            changed = False
            for inst in il:
                si = inst.sync_info
                if si is None:
                    out.append(inst)
                    continue
                waits = list(si.on_wait or [])
                updates = list(si.on_update or [])
                pre, post = [], []
                if len(waits) > max_waits:
                    for w in waits[max_waits:]:
                        nop = mybir.InstNoOp(name=fresh(), ins=[], outs=[])
                        nop.engine = inst.engine
                        nop.sync_info = mybir.SyncInfo(on_wait=[w], on_update=[])
                        pre.append(nop)
                    waits = waits[:max_waits]
                if len(updates) > max_updates:
                    for u in updates[max_updates:]:
                        nop = mybir.InstNoOp(name=fresh(), ins=[], outs=[])
                        nop.engine = inst.engine
                        nop.sync_info = mybir.SyncInfo(on_wait=[], on_update=[u])
                        post.append(nop)
                    updates = updates[:max_updates]
                if pre or post:
                    inst.sync_info = mybir.SyncInfo(on_wait=waits, on_update=updates)
                    changed = True
                out.extend(pre)
                out.append(inst)
                out.extend(post)
            if changed:
                blk.instructions = out


def _ln_to_transposed(nc, pools, src_ext, n_rows, scratch, g_sb, b_sb, eps_sb):
    """LayerNorm rows of src_ext [n_rows, DIM] f32 and return the
    normalized+affine result transposed as 8 bf16 tiles [128, n_rows]
    (partition = d within d-tile). Routes through a DRAM scratch so the
    transpose is 8 large DMA-transposes instead of n_rows/128*8 small
    PE transposes."""
    xin, stats, xhat_pool, singles = (
        pools["xin"],
        pools["stats"],
        pools["xhat"],
        pools["singles"],
    )
    n_t = n_rows // 128
    for t in range(n_t):
        xt = xin.tile([128, DIM], F32, tag="xin")
        nc.sync.dma_start(out=xt[:], in_=src_ext[t * 128 : (t + 1) * 128, :])
        st = stats.tile([128, 2, 6], F32, tag="bnst")
        xg = xt.rearrange("p (s d) -> p s d", s=2)
        nc.vector.bn_stats(out=st[:, 0, :], in_=xg[:, 0, :])
        nc.vector.bn_stats(out=st[:, 1, :], in_=xg[:, 1, :])
        mv = stats.tile([128, 2], F32, tag="bnmv")
        nc.vector.bn_aggr(out=mv[:], in_=st[:])
        stdev = stats.tile([128, 1], F32, tag="stdev")
        nc.scalar.activation(
            out=stdev[:],
            in_=mv[:, 1:2],
            func=mybir.ActivationFunctionType.Sqrt,
            bias=eps_sb[:],
            scale=1.0,
        )
        rstd = stats.tile([128, 1], F32, tag="rstd")
        nc.vector.reciprocal(out=rstd[:], in_=stdev[:])
        nmr = stats.tile([128, 1], F32, tag="nmr")
        nc.vector.tensor_scalar(
            out=nmr[:],
            in0=mv[:, 0:1],
            scalar1=rstd[:],
            scalar2=-1.0,
            op0=mybir.AluOpType.mult,
            op1=mybir.AluOpType.mult,
        )
        xh = xhat_pool.tile([128, DIM], BF16, tag="xhat")
        nc.vector.tensor_scalar(
            out=xh[:],
            in0=xt[:],
            scalar1=rstd[:],
            scalar2=nmr[:],
            op0=mybir.AluOpType.mult,
            op1=mybir.AluOpType.add,
        )
        nc.sync.dma_start(out=scratch[t * 128 : (t + 1) * 128, :], in_=xh[:])

    xT = []
    for kd in range(8):
        xt_t = singles.tile([128, n_rows], BF16, tag=f"xT_{scratch.name}_{kd}")
        nc.sync.dma_start_transpose(
            out=xt_t[:], in_=scratch[:, kd * 128 : (kd + 1) * 128]
        )
        # ln_g / ln_b are per-d == per-partition in this layout.
        nc.vector.tensor_scalar(
            out=xt_t[:],
            in0=xt_t[:],
            scalar1=g_sb[:, kd : kd + 1],
            scalar2=b_sb[:, kd : kd + 1],
            op0=mybir.AluOpType.mult,
            op1=mybir.AluOpType.add,
        )
        xT.append(xt_t)
    return xT


def build_graph(l_kv):
    """One SPMD graph; per-core tensors differ only in content."""
    lt = l_kv // 128  # k tiles
    nc = bass.Bass()

    x_ext = nc.declare_dram_parameter("x", [N_TOK, DIM], F32, isOutput=False)
    xkv_ext = (
        nc.declare_dram_parameter("xkv", [l_kv, DIM], F32, isOutput=False)
        if COMPACT
        else None
    )
    gate_ext = nc.declare_dram_parameter("gate", [l_kv], F32, isOutput=False)
    wq_ext = nc.declare_dram_parameter("wq", [DIM, COLS], F32, isOutput=False)
    wk_ext = nc.declare_dram_parameter("wk", [DIM, COLS], F32, isOutput=False)
    wv_ext = nc.declare_dram_parameter("wv", [DIM, COLS], F32, isOutput=False)
    g_ext = nc.declare_dram_parameter("ln_g", [DIM], F32, isOutput=False)
    b_ext = nc.declare_dram_parameter("ln_b", [DIM], F32, isOutput=False)
    out_ext = nc.declare_dram_parameter("out", [N_TOK, COLS], F32, isOutput=True)

    xhat_q_scratch = nc.dram_tensor("xhat_q_scratch", [N_TOK, DIM], BF16)
    xhat_kv_scratch = (
        nc.dram_tensor("xhat_kv_scratch", [l_kv, DIM], BF16) if COMPACT else None
    )

    with tile.TileContext(nc) as tc:
        import contextlib

        with contextlib.ExitStack() as ctx:
            singles = ctx.enter_context(tc.tile_pool(name="singles", bufs=1))
            pools = {
                "singles": singles,
                "xin": ctx.enter_context(tc.tile_pool(name="xin", bufs=3)),
                "stats": ctx.enter_context(tc.tile_pool(name="stats", bufs=3)),
                "xhat": ctx.enter_context(tc.tile_pool(name="xhat", bufs=3)),
                "wtmp": ctx.enter_context(tc.tile_pool(name="wtmp", bufs=2)),
            }

            # --- constants -------------------------------------------------
            g_sb = singles.tile([128, 8], F32, tag="g_sb")
            nc.sync.dma_start(out=g_sb[:], in_=g_ext.rearrange("(kd p) -> p kd", p=128))
            b_sb = singles.tile([128, 8], F32, tag="b_sb")
            nc.sync.dma_start(out=b_sb[:], in_=b_ext.rearrange("(kd p) -> p kd", p=128))
            gate_sb = singles.tile([128, lt], F32, tag="gate_sb")
            nc.sync.dma_start(
                out=gate_sb[:], in_=gate_ext.rearrange("(t p) -> p t", p=128)
            )
            eps_sb = singles.tile([128, 1], F32, tag="eps_sb")
            nc.vector.memset(eps_sb[:], EPS)
            ident = singles.tile([128, 128], F32, tag="ident")
            from concourse.masks import make_identity

            make_identity(nc, ident[:])

            # --- weights: f32 -> bf16 -------------------------------------
            wg = {}
            for name, ext in (("q", wq_ext), ("k", wk_ext), ("v", wv_ext)):
                tiles = []
                for kd in range(8):
                    wt = pools["wtmp"].tile([128, COLS], F32, tag="wtmp")
                    nc.sync.dma_start(
                        out=wt[:], in_=ext[kd * 128 : (kd + 1) * 128, :]
                    )
                    wb = singles.tile([128, COLS], BF16, tag=f"wg_{name}_{kd}")
                    nc.vector.tensor_copy(wb[:], wt[:])
                    tiles.append(wb)
                wg[name] = tiles

            # --- LN + transpose -------------------------------------------
            xqT = _ln_to_transposed(
                nc, pools, x_ext, N_TOK, xhat_q_scratch, g_sb, b_sb, eps_sb
            )
            if COMPACT:
                xkvT = _ln_to_transposed(
                    nc, pools, xkv_ext, l_kv, xhat_kv_scratch, g_sb, b_sb, eps_sb
                )
            else:
                xkvT = xqT  # dense: kv tokens are the q tokens

            # --- projections ----------------------------------------------
            with tc.tile_pool(name="psum_proj", bufs=4, space="PSUM") as psum_proj:
                qT = []
                for cb in range(4):
                    qt = singles.tile([128, N_TOK], BF16, tag=f"qT_{cb}")
                    for tcn in range(N_TOK // 512):
                        ps = psum_proj.tile([128, 512], F32, tag="proj")
                        for kd in range(8):
                            nc.tensor.matmul(
                                ps[:],
                                wg["q"][kd][:, cb * 128 : (cb + 1) * 128],
                                xqT[kd][:, tcn * 512 : (tcn + 1) * 512],
                                start=(kd == 0),
                                stop=(kd == 7),
                            )
                        nc.vector.tensor_copy(
                            qt[:, tcn * 512 : (tcn + 1) * 512], ps[:]
                        )
                    qT.append(qt)

                kT = []
                kchunks = []
                off = 0
                while off < l_kv:
                    sz = min(512, l_kv - off)
                    kchunks.append((off, sz))
                    off += sz
                for cb in range(4):
                    kt = singles.tile([128, l_kv], BF16, tag=f"kT_{cb}")
                    for off, sz in kchunks:
                        ps = psum_proj.tile([128, 512], F32, tag="proj")
                        for kd in range(8):
                            nc.tensor.matmul(
                                ps[:, :sz],
                                wg["k"][kd][:, cb * 128 : (cb + 1) * 128],
                                xkvT[kd][:, off : off + sz],
                                start=(kd == 0),
                                stop=(kd == 7),
                            )
                        nc.vector.tensor_copy(kt[:, off : off + sz], ps[:, :sz])
                    kT.append(kt)

                vaug = []
                for tb in range(lt):
                    va = singles.tile([128, HEADS_LOCAL * 65], BF16, tag=f"vaug_{tb}")
                    ps = psum_proj.tile([128, COLS], F32, tag="proj")
                    for kd in range(8):
                        nc.tensor.matmul(
                            ps[:],
                            xkvT[kd][:, tb * 128 : (tb + 1) * 128],
                            wg["v"][kd][:],
                            start=(kd == 0),
                            stop=(kd == 7),
                        )
                    for h in range(HEADS_LOCAL):
                        nc.vector.tensor_scalar(
                            out=va[:, h * 65 : h * 65 + 64],
                            in0=ps[:, h * 64 : (h + 1) * 64],
                            scalar1=gate_sb[:, tb : tb + 1],
                            scalar2=None,
                            op0=mybir.AluOpType.mult,
                        )
                        nc.vector.tensor_copy(
                            va[:, h * 65 + 64 : h * 65 + 65], gate_sb[:, tb : tb + 1]
                        )
                    vaug.append(va)

            # --- attention -------------------------------------------------
            ngroups = (lt + KGROUP - 1) // KGROUP
            with (
                tc.tile_pool(name="psum_s", bufs=2, space="PSUM") as psum_s_pool,
                tc.tile_pool(name="psum_o", bufs=1, space="PSUM") as psum_o_pool,
                tc.tile_pool(name="psum_t", bufs=1, space="PSUM") as psum_t_pool,
                tc.tile_pool(name="p_sb", bufs=2) as p_pool,
                tc.tile_pool(name="o_sb", bufs=2) as o_pool,
                tc.tile_pool(name="outtiles", bufs=2) as out_pool,
                tc.tile_pool(name="recip", bufs=2) as recip_pool,
            ):
                for qc in range(N_TOK // QCHUNK):
                    out_tiles = [
                        out_pool.tile(
                            [128, COLS], F32, tag=f"out_{j}", name=f"out_{qc}_{j}"
                        )
                        for j in range(4)
                    ]
                    for h in range(HEADS_LOCAL):
                        cb, p0 = h // 2, (h % 2) * 64
                        psum_o = psum_o_pool.tile([65, 512], F32, tag="o")
                        for gi in range(ngroups):
                            gsz = min(KGROUP, lt - gi * KGROUP)
                            psum_s = psum_s_pool.tile([128, KGROUP * 512], F32, tag="s")
                            for i in range(gsz):
                                tb = gi * KGROUP + i
                                nc.tensor.matmul(
                                    psum_s[:, i * 512 : (i + 1) * 512],
                                    kT[cb][p0 : p0 + 64, tb * 128 : (tb + 1) * 128],
                                    qT[cb][p0 : p0 + 64, qc * 512 : (qc + 1) * 512],
                                    start=True,
                                    stop=True,
                                )
                            p_sb = p_pool.tile([128, KGROUP * 512], BF16, tag="p")
                            nc.scalar.activation(
                                out=p_sb[:, : gsz * 512],
                                in_=psum_s[:, : gsz * 512],
                                func=mybir.ActivationFunctionType.Exp,
                                scale=SCALE,
                            )
                            for i in range(gsz):
                                tb = gi * KGROUP + i
                                nc.tensor.matmul(
                                    psum_o[:],
                                    vaug[tb][:, h * 65 : (h + 1) * 65],
                                    p_sb[:, i * 512 : (i + 1) * 512],
                                    start=(tb == 0),
                                    stop=(tb == lt - 1),
                                )
                        o_sb = o_pool.tile([65, 512], F32, tag="o_sb")
                        nc.vector.tensor_copy(o_sb[:], psum_o[:])
                        psum_t = psum_t_pool.tile([128, 4 * 65], F32, tag="t")
                        for j in range(4):
                            nc.tensor.transpose(
                                psum_t[:, j * 65 : (j + 1) * 65],
                                o_sb[:, j * 128 : (j + 1) * 128],
                                ident[0:65, 0:65],
                            )
                        recip = recip_pool.tile([128, 4], F32, tag="recip")
                        denom = psum_t.rearrange("p (j c) -> p j c", c=65)[:, :, 64:65]
                        nc.vector.reciprocal(out=recip[:], in_=denom)
                        for j in range(4):
                            nc.vector.tensor_scalar(
                                out=out_tiles[j][:, h * 64 : (h + 1) * 64],
                                in0=psum_t[:, j * 65 : j * 65 + 64],
                                scalar1=recip[:, j : j + 1],
                                scalar2=None,
                                op0=mybir.AluOpType.mult,
                            )
                    for j in range(4):
                        row0 = qc * QCHUNK + j * 128
                        nc.sync.dma_start(
                            out=out_ext[row0 : row0 + 128, :], in_=out_tiles[j][:]
                        )

    _split_excess_waits(nc)
    return nc


_GRAPH_CACHE = {}


def kernel(x, mask, w_qkv, ln_g, ln_b):
    x = np.asarray(x, dtype=np.float32)
    mask = np.asarray(mask)
    w_qkv = np.asarray(w_qkv, dtype=np.float32)
    ln_g = np.asarray(ln_g, dtype=np.float32)
    ln_b = np.asarray(ln_b, dtype=np.float32)
    b, n, d = x.shape

    if COMPACT:
        keeps = [np.where(mask[bi] == 0)[0] for bi in range(b)]
        l_kv = max(128, -(-max(len(k) for k in keeps) // 128) * 128)
    else:
        keeps = None
        l_kv = n

    global LAST_EXEC_TIME_NS
    key = (l_kv, COMPACT)
    if key not in _GRAPH_CACHE:
        _GRAPH_CACHE[key] = build_graph(l_kv)
    nc = _GRAPH_CACHE[key]

    in_maps = []
    for core in range(N_CORES):
        bi, hg = core // 2, core % 2
        m = {
            "x": x[bi],
            "wq": np.ascontiguousarray(w_qkv[:, hg * COLS : (hg + 1) * COLS]),
            "wk": np.ascontiguousarray(w_qkv[:, d + hg * COLS : d + (hg + 1) * COLS]),
            "wv": np.ascontiguousarray(
                w_qkv[:, 2 * d + hg * COLS : 2 * d + (hg + 1) * COLS]
            ),
            "ln_g": ln_g,
            "ln_b": ln_b,
        }
        if COMPACT:
            keep = keeps[bi]
            xkv = np.zeros((l_kv, d), dtype=np.float32)
            xkv[: len(keep)] = x[bi][keep]
            gate = np.zeros((l_kv,), dtype=np.float32)
            gate[: len(keep)] = 1.0
            m["xkv"] = xkv
            m["gate"] = gate
        else:
            m["gate"] = (1.0 - mask[bi].astype(np.float32))
        in_maps.append(m)

    from concourse.bass_utils import run_bass_kernel_spmd

    trace = os.environ.get("KERNEL_TRACE", "") == "1"
    kwargs = {}
    if trace:
        import antenv

        if "antenv.axon_hooks" not in sys.modules:
            hooks = types.ModuleType("antenv.axon_hooks")
            hooks._hook = None
            hooks.set_axon_ntff_profile_hook = lambda h: setattr(hooks, "_hook", h)
            hooks.get_axon_ntff_profile_hook = lambda: hooks._hook
            sys.modules["antenv.axon_hooks"] = hooks
            antenv.axon_hooks = hooks
        from trn_agent_boot.trn_boot import _ntff_profile_via_ctypes

        sys.modules["antenv.axon_hooks"].set_axon_ntff_profile_hook(
            _ntff_profile_via_ctypes("/opt/axon/libaxon_pjrt.so")
        )
        from concourse import bass_utils

        bass_utils.upload_artifacts = lambda tmpdir: tmpdir
        tdir = os.environ.get("KERNEL_TRACE_DIR", "/tmp/kernel_trace")
        os.makedirs(tdir, exist_ok=True)
        kwargs = {"trace": True, "tmpdir": tdir}

    res = run_bass_kernel_spmd(nc, in_maps, core_ids=list(range(N_CORES)), **kwargs)
    LAST_EXEC_TIME_NS = res.exec_time_ns

    out = np.empty((b, n, d), dtype=np.float32)
    for core in range(N_CORES):
        bi, hg = core // 2, core % 2
        out[bi][:, hg * COLS : (hg + 1) * COLS] = res.results[core]["out"]
    return out


# revision 4
# speedup vs baseline: 1.3678x; 1.3678x over previous
"""Distributed Trainium2 kernel for nn_Attention_49529562858354.

Reference computation (per batch): LayerNorm(x) @ w_qkv -> 16-head
self-attention with key-side masking (mask==1 -> key excluded).

Sharding (8 cores): core = batch * 2 + head_group. Data parallel over
the 4 batches, tensor parallel over 2 groups of 8 heads. Each core gets
its batch's x, the w_qkv column slice for its heads, and produces
out[:, hg*512:(hg+1)*512] for its batch. No collectives needed.

Per-core pipeline:
  1. LayerNorm stats in natural layout (bn_stats), xhat -> bf16.
  2. xhat -> DRAM scratch -> 8 DMA-transposes -> xT [128d x 8, tokens].
     ln_g/ln_b applied per-partition on xT (d is the partition dim there).
  3. QKV projections in bf16: qT/kT as [cols, tokens] (transposed),
     v in natural [tokens, cols] layout with the key gate folded in and
     a gate column appended (softmax denominator via matmul).
  4. Attention per (head, q-chunk of 512): scores computed transposed
     [k_tile=128, 512q] into PSUM, exp on the Scalar engine directly
     from PSUM (scale fused, no max subtraction needed: logits are O(1)
     after LN), AV accumulates outT_aug [65, 512] whose row 64 is the
     softmax denominator. Epilogue: PE transpose, reciprocal, scale,
     assemble [128, 512] output tiles, DMA out.

Masked keys are removed on the host (gather) and the remainder padded
to a multiple of 128 with gate=0 rows, halving attention work; the same
kernel runs dense (all 2048 keys, gate = 1-mask) when COMPACT=False.
"""

import os
import sys
import types

for _p in ("/opt/trn_rl_repo", "/root/.axon_site"):
    if _p not in sys.path:
        sys.path.insert(0, _p)

import numpy as np
import ml_dtypes

import concourse.bass as bass
import concourse.tile as tile
from concourse import mybir

N_CORES = 8
N_TOK = 2048
DIM = 1024
HEADS_LOCAL = 8
DH = 64
COLS = HEADS_LOCAL * DH  # 512 columns per core per q/k/v
SCALE = DH ** -0.5
EPS = 1e-5
QCHUNK = 512
KGROUP = 3  # score k-tiles per PSUM group / exp call
COMPACT = os.environ.get("KERNEL_DENSE", "") != "1"

F32 = mybir.dt.float32
BF16 = mybir.dt.bfloat16

LAST_EXEC_TIME_NS = None


def _split_excess_waits(nc, max_waits=1, max_updates=1):
    """This container's walrus rejects >1 sync wait/update per
    instruction; move overflow onto adjacent same-engine NoOps."""
    counter = [0]

    def fresh():
        counter[0] += 1
        return f"I-WFIX-{counter[0]}"

    for f in nc.m.functions:
        for blk in f.blocks:
            il = blk.instructions
            out = []
            changed = False
            for inst in il:
                si = inst.sync_info
                if si is None:
                    out.append(inst)
                    continue
                waits = list(si.on_wait or [])
                updates = list(si.on_update or [])
                pre, post = [], []
                if len(waits) > max_waits:
                    for w in waits[max_waits:]:
                        nop = mybir.InstNoOp(name=fresh(), ins=[], outs=[])
                        nop.engine = inst.engine
                        nop.sync_info = mybir.SyncInfo(on_wait=[w], on_update=[])
                        pre.append(nop)
                    waits = waits[:max_waits]
                if len(updates) > max_updates:
                    for u in updates[max_updates:]:
                        nop = mybir.InstNoOp(name=fresh(), ins=[], outs=[])
                        nop.engine = inst.engine
                        nop.sync_info = mybir.SyncInfo(on_wait=[], on_update=[u])
                        post.append(nop)
                    updates = updates[:max_updates]
                if pre or post:
                    inst.sync_info = mybir.SyncInfo(on_wait=waits, on_update=updates)
                    changed = True
                out.extend(pre)
                out.append(inst)
                out.extend(post)
            if changed:
                blk.instructions = out


def _ln_to_transposed(nc, pools, src_ext, n_rows, scratch, g_sb, b_sb, eps_sb):
    """LayerNorm rows of src_ext [n_rows, DIM] f32 and return the
    normalized+affine result transposed as 8 bf16 tiles [128, n_rows]
    (partition = d within d-tile). Routes through a DRAM scratch so the
    transpose is 8 large DMA-transposes instead of n_rows/128*8 small
    PE transposes."""
    xin, stats, xhat_pool, singles = (
        pools["xin"],
        pools["stats"],
        pools["xhat"],
        pools["singles"],
    )
    n_t = n_rows // 128
    for t in range(n_t):
        xt = xin.tile([128, DIM], F32, tag="xin")
        nc.sync.dma_start(out=xt[:], in_=src_ext[t * 128 : (t + 1) * 128, :])
        st = stats.tile([128, 2, 6], F32, tag="bnst")
        xg = xt.rearrange("p (s d) -> p s d", s=2)
        nc.vector.bn_stats(out=st[:, 0, :], in_=xg[:, 0, :])
        nc.vector.bn_stats(out=st[:, 1, :], in_=xg[:, 1, :])
        mv = stats.tile([128, 2], F32, tag="bnmv")
        nc.vector.bn_aggr(out=mv[:], in_=st[:])
        stdev = stats.tile([128, 1], F32, tag="stdev")
        nc.scalar.activation(
            out=stdev[:],
            in_=mv[:, 1:2],
            func=mybir.ActivationFunctionType.Sqrt,
            bias=eps_sb[:],
            scale=1.0,
        )
        rstd = stats.tile([128, 1], F32, tag="rstd")
        nc.vector.reciprocal(out=rstd[:], in_=stdev[:])
        nmr = stats.tile([128, 1], F32, tag="nmr")
        nc.vector.tensor_scalar(
            out=nmr[:],
            in0=mv[:, 0:1],
            scalar1=rstd[:],
            scalar2=-1.0,
            op0=mybir.AluOpType.mult,
            op1=mybir.AluOpType.mult,
        )
        xh = xhat_pool.tile([128, DIM], BF16, tag="xhat")
        nc.vector.tensor_scalar(
            out=xh[:],
            in0=xt[:],
            scalar1=rstd[:],
            scalar2=nmr[:],
            op0=mybir.AluOpType.mult,
            op1=mybir.AluOpType.add,
        )
        nc.sync.dma_start(out=scratch[t * 128 : (t + 1) * 128, :], in_=xh[:])

    xT = []
    for kd in range(8):
        xt_t = singles.tile([128, n_rows], BF16, tag=f"xT_{scratch.name}_{kd}")
        nc.sync.dma_start_transpose(
            out=xt_t[:], in_=scratch[:, kd * 128 : (kd + 1) * 128]
        )
        # ln_g / ln_b are per-d == per-partition in this layout.
        nc.vector.tensor_scalar(
            out=xt_t[:],
            in0=xt_t[:],
            scalar1=g_sb[:, kd : kd + 1],
            scalar2=b_sb[:, kd : kd + 1],
            op0=mybir.AluOpType.mult,
            op1=mybir.AluOpType.add,
        )
        xT.append(xt_t)
    return xT


def build_graph(l_kv):
    """One SPMD graph; per-core tensors differ only in content."""
    lt = l_kv // 128  # k tiles
    nc = bass.Bass()

    x_ext = nc.declare_dram_parameter("x", [N_TOK, DIM], F32, isOutput=False)
    xkv_ext = (
        nc.declare_dram_parameter("xkv", [l_kv, DIM], F32, isOutput=False)
        if COMPACT
        else None
    )
    gate_ext = nc.declare_dram_parameter("gate", [l_kv], F32, isOutput=False)
    wq_ext = nc.declare_dram_parameter("wq", [DIM, COLS], F32, isOutput=False)
    wk_ext = nc.declare_dram_parameter("wk", [DIM, COLS], F32, isOutput=False)
    wv_ext = nc.declare_dram_parameter("wv", [DIM, COLS], F32, isOutput=False)
    g_ext = nc.declare_dram_parameter("ln_g", [DIM], F32, isOutput=False)
    b_ext = nc.declare_dram_parameter("ln_b", [DIM], F32, isOutput=False)
    out_ext = nc.declare_dram_parameter("out", [N_TOK, COLS], F32, isOutput=True)

    xhat_q_scratch = nc.dram_tensor("xhat_q_scratch", [N_TOK, DIM], BF16)
    xhat_kv_scratch = (
        nc.dram_tensor("xhat_kv_scratch", [l_kv, DIM], BF16) if COMPACT else None
    )

    with tile.TileContext(nc) as tc:
        import contextlib

        with contextlib.ExitStack() as ctx:
            singles = ctx.enter_context(tc.tile_pool(name="singles", bufs=1))
            pools = {
                "singles": singles,
                "xin": ctx.enter_context(tc.tile_pool(name="xin", bufs=3)),
                "stats": ctx.enter_context(tc.tile_pool(name="stats", bufs=3)),
                "xhat": ctx.enter_context(tc.tile_pool(name="xhat", bufs=3)),
                "wtmp": ctx.enter_context(tc.tile_pool(name="wtmp", bufs=2)),
            }

            # --- constants -------------------------------------------------
            g_sb = singles.tile([128, 8], F32, tag="g_sb")
            nc.sync.dma_start(out=g_sb[:], in_=g_ext.rearrange("(kd p) -> p kd", p=128))
            b_sb = singles.tile([128, 8], F32, tag="b_sb")
            nc.sync.dma_start(out=b_sb[:], in_=b_ext.rearrange("(kd p) -> p kd", p=128))
            gate_sb = singles.tile([128, lt], F32, tag="gate_sb")
            nc.sync.dma_start(
                out=gate_sb[:], in_=gate_ext.rearrange("(t p) -> p t", p=128)
            )
            eps_sb = singles.tile([128, 1], F32, tag="eps_sb")
            nc.vector.memset(eps_sb[:], EPS)
            ident = singles.tile([128, 128], F32, tag="ident")
            from concourse.masks import make_identity

            make_identity(nc, ident[:])

            # --- weights: f32 -> bf16 -------------------------------------
            wg = {}
            for name, ext in (("q", wq_ext), ("k", wk_ext), ("v", wv_ext)):
                tiles = []
                for kd in range(8):
                    wt = pools["wtmp"].tile([128, COLS], F32, tag="wtmp")
                    nc.sync.dma_start(
                        out=wt[:], in_=ext[kd * 128 : (kd + 1) * 128, :]
                    )
                    wb = singles.tile([128, COLS], BF16, tag=f"wg_{name}_{kd}")
                    nc.vector.tensor_copy(wb[:], wt[:])
                    tiles.append(wb)
                wg[name] = tiles

            # --- LN + transpose -------------------------------------------
            xqT = _ln_to_transposed(
                nc, pools, x_ext, N_TOK, xhat_q_scratch, g_sb, b_sb, eps_sb
            )
            if COMPACT:
                xkvT = _ln_to_transposed(
                    nc, pools, xkv_ext, l_kv, xhat_kv_scratch, g_sb, b_sb, eps_sb
                )
            else:
                xkvT = xqT  # dense: kv tokens are the q tokens

            # --- projections ----------------------------------------------
            with tc.tile_pool(name="psum_proj", bufs=4, space="PSUM") as psum_proj:
                qT = []
                for cb in range(4):
                    qt = singles.tile([128, N_TOK], BF16, tag=f"qT_{cb}")
                    for tcn in range(N_TOK // 512):
                        ps = psum_proj.tile([128, 512], F32, tag="proj")
                        for kd in range(8):
                            nc.tensor.matmul(
                                ps[:],
                                wg["q"][kd][:, cb * 128 : (cb + 1) * 128],
                                xqT[kd][:, tcn * 512 : (tcn + 1) * 512],
                                start=(kd == 0),
                                stop=(kd == 7),
                            )
                        nc.vector.tensor_copy(
                            qt[:, tcn * 512 : (tcn + 1) * 512], ps[:]
                        )
                    qT.append(qt)

                kT = []
                kchunks = []
                off = 0
                while off < l_kv:
                    sz = min(512, l_kv - off)
                    kchunks.append((off, sz))
                    off += sz
                for cb in range(4):
                    kt = singles.tile([128, l_kv], BF16, tag=f"kT_{cb}")
                    for off, sz in kchunks:
                        ps = psum_proj.tile([128, 512], F32, tag="proj")
                        for kd in range(8):
                            nc.tensor.matmul(
                                ps[:, :sz],
                                wg["k"][kd][:, cb * 128 : (cb + 1) * 128],
                                xkvT[kd][:, off : off + sz],
                                start=(kd == 0),
                                stop=(kd == 7),
                            )
                        nc.vector.tensor_copy(kt[:, off : off + sz], ps[:, :sz])
                    kT.append(kt)

                vaug = []
                for tb in range(lt):
                    va = singles.tile([128, HEADS_LOCAL * 65], BF16, tag=f"vaug_{tb}")
                    ps = psum_proj.tile([128, COLS], F32, tag="proj")
                    for kd in range(8):
                        nc.tensor.matmul(
                            ps[:],
                            xkvT[kd][:, tb * 128 : (tb + 1) * 128],
                            wg["v"][kd][:],
                            start=(kd == 0),
                            stop=(kd == 7),
                        )
                    for h in range(HEADS_LOCAL):
                        nc.vector.tensor_scalar(
                            out=va[:, h * 65 : h * 65 + 64],
                            in0=ps[:, h * 64 : (h + 1) * 64],
                            scalar1=gate_sb[:, tb : tb + 1],
                            scalar2=None,
                            op0=mybir.AluOpType.mult,
                        )
                        nc.vector.tensor_copy(
                            va[:, h * 65 + 64 : h * 65 + 65], gate_sb[:, tb : tb + 1]
                        )
                    vaug.append(va)

            # --- attention -------------------------------------------------
            ngroups = (lt + KGROUP - 1) // KGROUP
            with (
                tc.tile_pool(name="psum_s", bufs=2, space="PSUM") as psum_s_pool,
                tc.tile_pool(name="psum_o", bufs=1, space="PSUM") as psum_o_pool,
                tc.tile_pool(name="psum_t", bufs=1, space="PSUM") as psum_t_pool,
                tc.tile_pool(name="p_sb", bufs=2) as p_pool,
                tc.tile_pool(name="o_sb", bufs=2) as o_pool,
                tc.tile_pool(name="outtiles", bufs=2) as out_pool,
                tc.tile_pool(name="recip", bufs=2) as recip_pool,
            ):
                for qc in range(N_TOK // QCHUNK):
                    out_tiles = [
                        out_pool.tile(
                            [128, COLS], F32, tag=f"out_{j}", name=f"out_{qc}_{j}"
                        )
                        for j in range(4)
                    ]
                    for h in range(HEADS_LOCAL):
                        cb, p0 = h // 2, (h % 2) * 64
                        psum_o = psum_o_pool.tile([65, 512], F32, tag="o")
                        for gi in range(ngroups):
                            gsz = min(KGROUP, lt - gi * KGROUP)
                            psum_s = psum_s_pool.tile([128, KGROUP * 512], F32, tag="s")
                            for i in range(gsz):
                                tb = gi * KGROUP + i
                                nc.tensor.matmul(
                                    psum_s[:, i * 512 : (i + 1) * 512],
                                    kT[cb][p0 : p0 + 64, tb * 128 : (tb + 1) * 128],
                                    qT[cb][p0 : p0 + 64, qc * 512 : (qc + 1) * 512],
                                    start=True,
                                    stop=True,
                                )
                            p_sb = p_pool.tile([128, KGROUP * 512], BF16, tag="p")
                            nc.scalar.activation(
                                out=p_sb[:, : gsz * 512],
                                in_=psum_s[:, : gsz * 512],
                                func=mybir.ActivationFunctionType.Exp,
                                scale=SCALE,
                            )
                            for i in range(gsz):
                                tb = gi * KGROUP + i
                                nc.tensor.matmul(
                                    psum_o[:],
                                    vaug[tb][:, h * 65 : (h + 1) * 65],
                                    p_sb[:, i * 512 : (i + 1) * 512],
                                    start=(tb == 0),
                                    stop=(tb == lt - 1),
                                )
                        o_sb = o_pool.tile([65, 512], F32, tag="o_sb")
                        nc.vector.tensor_copy(o_sb[:], psum_o[:])
                        psum_t = psum_t_pool.tile([128, 4 * 65], F32, tag="t")
                        for j in range(4):
                            nc.tensor.transpose(
                                psum_t[:, j * 65 : (j + 1) * 65],
                                o_sb[:, j * 128 : (j + 1) * 128],
                                ident[0:65, 0:65],
                            )
                        recip = recip_pool.tile([128, 4], F32, tag="recip")
                        denom = psum_t.rearrange("p (j c) -> p j c", c=65)[:, :, 64:65]
                        nc.vector.reciprocal(out=recip[:], in_=denom)
                        for j in range(4):
                            nc.vector.tensor_scalar(
                                out=out_tiles[j][:, h * 64 : (h + 1) * 64],
                                in0=psum_t[:, j * 65 : j * 65 + 64],
                                scalar1=recip[:, j : j + 1],
                                scalar2=None,
                                op0=mybir.AluOpType.mult,
                            )
                    for j in range(4):
                        row0 = qc * QCHUNK + j * 128
                        nc.sync.dma_start(
                            out=out_ext[row0 : row0 + 128, :], in_=out_tiles[j][:]
                        )

    _split_excess_waits(nc)
    return nc


_GRAPH_CACHE = {}


def kernel(x, mask, w_qkv, ln_g, ln_b):
    x = np.asarray(x, dtype=np.float32)
    mask = np.asarray(mask)
    w_qkv = np.asarray(w_qkv, dtype=np.float32)
    ln_g = np.asarray(ln_g, dtype=np.float32)
    ln_b = np.asarray(ln_b, dtype=np.float32)
    b, n, d = x.shape

    if COMPACT:
        keeps = [np.where(mask[bi] == 0)[0] for bi in range(b)]
        l_kv = max(128, -(-max(len(k) for k in keeps) // 128) * 128)
    else:
        keeps = None
        l_kv = n

    global LAST_EXEC_TIME_NS
    key = (l_kv, COMPACT)
    if key not in _GRAPH_CACHE:
        _GRAPH_CACHE[key] = build_graph(l_kv)
    nc = _GRAPH_CACHE[key]

    in_maps = []
    for core in range(N_CORES):
        bi, hg = core // 2, core % 2
        m = {
            "x": x[bi],
            "wq": np.ascontiguousarray(w_qkv[:, hg * COLS : (hg + 1) * COLS]),
            "wk": np.ascontiguousarray(w_qkv[:, d + hg * COLS : d + (hg + 1) * COLS]),
            "wv": np.ascontiguousarray(
                w_qkv[:, 2 * d + hg * COLS : 2 * d + (hg + 1) * COLS]
            ),
            "ln_g": ln_g,
            "ln_b": ln_b,
        }
        if COMPACT:
            keep = keeps[bi]
            xkv = np.zeros((l_kv, d), dtype=np.float32)
            xkv[: len(keep)] = x[bi][keep]
            gate = np.zeros((l_kv,), dtype=np.float32)
            gate[: len(keep)] = 1.0
            m["xkv"] = xkv
            m["gate"] = gate
        else:
            m["gate"] = (1.0 - mask[bi].astype(np.float32))
        in_maps.append(m)

    from concourse.bass_utils import run_bass_kernel_spmd

    trace = os.environ.get("KERNEL_TRACE", "") == "1"
    kwargs = {}
    if trace:
        import antenv

        if "antenv.axon_hooks" not in sys.modules:
            hooks = types.ModuleType("antenv.axon_hooks")
            hooks._hook = None
            hooks.set_axon_ntff_profile_hook = lambda h: setattr(hooks, "_hook", h)
            hooks.get_axon_ntff_profile_hook = lambda: hooks._hook
            sys.modules["antenv.axon_hooks"] = hooks
            antenv.axon_hooks = hooks
        from trn_agent_boot.trn_boot import _ntff_profile_via_ctypes

        sys.modules["antenv.axon_hooks"].set_axon_ntff_profile_hook(
            _ntff_profile_via_ctypes("/opt/axon/libaxon_pjrt.so")
        )
        from concourse import bass_utils

        bass_utils.upload_artifacts = lambda tmpdir: tmpdir
        import uuid

        tdir = os.path.join(
            os.environ.get("KERNEL_TRACE_DIR", "/tmp/kernel_trace"), uuid.uuid4().hex[:8]
        )
        os.makedirs(tdir, exist_ok=True)
        kwargs = {"trace": True, "tmpdir": tdir}

    res = run_bass_kernel_spmd(nc, in_maps, core_ids=list(range(N_CORES)), **kwargs)
    LAST_EXEC_TIME_NS = res.exec_time_ns

    out = np.empty((b, n, d), dtype=np.float32)
    for core in range(N_CORES):
        bi, hg = core // 2, core % 2
        out[bi][:, hg * COLS : (hg + 1) * COLS] = res.results[core]["out"]
    return out
